# revision 65
# baseline (speedup 1.0000x reference)
"""Additive attention (Bahdanau) Trainium2 kernel, SPMD over 8 NeuronCores.

Math per batch b (see reference):
    q = queries[b] @ Wq                  [Q=128, H=256]
    k = keys[b]    @ Wk                  [K=1024, H=256]
    scores[i,j] = sum_h wv[h] * tanh(q[i,h] + k[j,h])
    attn = masked_softmax(scores, valid_len[b])
    out[b] = attn @ values[b]            [Q, V=512]

Sharding: sequence-parallel q-striping. Each core takes 16 q-rows of EVERY
batch and only the valid k-range of each batch (rounded up to 128). Per-core
work = sum_b 16*ceil(vl_b/128)*128 columns -- perfectly balanced for any
valid_lens, no collectives (softmax is per-q-row and stays core-local).

Device pipeline (per core), h-on-partitions layout, fully group-streamed:
  - per group g (one batch): DMA its kT slice, project kh_g (PE) + cast
    (DVE), broadcast-add q (DVE tensor_scalar, bf16 4x), tanh both h-tiles
    in ONE fused ACTIVATE per 8-row chunk (ACT is the critical engine at
    1 elem/cycle/lane), score rows via M=1 matmuls into PSUM slabs
    (quadrant tile_position), drain slab->scores_sb via stage copy +
    strided DMA
  - group g-1's epilogue (exp -> DMA-xbar transpose -> AV matmul into a
    single PSUM accumulator) is interleaved into group g's chunks so the
    scalar engine never waits
  - scores_sb pre-filled with -60 (GPSIMD memset) so cross-batch cells
    exp to ~0; no max subtraction (|scores| <= sum|wv| ~ 13)
  - mask baked into values_ext: rows k >= vl zeroed, last column 1[k < vl]
    so attn @ values_ext also yields the softmax denominator
  - out = av * recip(l)
"""

import os
import sys

import numpy as np

for _p in ("/opt/trn_rl_repo", "/root/.axon_site/_ro/trn_rl_repo"):
    if os.path.isdir(_p) and _p not in sys.path:
        sys.path.insert(0, _p)

os.environ.setdefault("MYCRO_LOCAL_CACHE", "1")

import ml_dtypes  # noqa: E402
from contextlib import ExitStack  # noqa: E402

import concourse.bass as bass  # noqa: E402
import concourse.tile as tile  # noqa: E402
from concourse import bacc, mybir  # noqa: E402
from concourse.bass_utils import run_bass_kernel_spmd  # noqa: E402
from concourse.masks import make_identity  # noqa: E402

BF16 = mybir.dt.bfloat16
F32 = mybir.dt.float32
NP_BF16 = ml_dtypes.bfloat16

B, Q, K, D, H, V = 8, 128, 1024, 512, 256, 512
DC = D // 128   # 4 contraction tiles for the projections
HT = H // 128   # 2 h-tiles
QPC = Q // B    # 16 q-rows per (batch, core)
VE = V + 1      # values extended with a ones column (softmax denominator)
NEG = -60.0     # filler for never-written score cells; exp(-60) ~ 9e-27


def _kw_template(valid_lens):
    """Group order: 2nd-smallest kw first (fast pipeline start), the rest
    descending, smallest last (short epilogue tail). Returns per-GROUP
    (batch index, kw, koff); everything (kT/v packing, q slots, scores
    columns) uses this order."""
    kwb = [min(K, max(128, int(-(-int(v) // 128) * 128))) for v in valid_lens]
    by_kw = sorted(range(len(kwb)), key=lambda b: (kwb[b], b))
    order = [by_kw[1]] + sorted(by_kw[2:], key=lambda b: (-kwb[b], b)) + [by_kw[0]]
    kw = [kwb[b] for b in order]
    koff = np.concatenate([[0], np.cumsum(kw)]).astype(int)
    # compute-stage widths on a 2 grid (S-build/tanh/score matmuls only --
    # even so DVE 4x packing applies); the epilogue (exp/transpose/AV)
    # stays on the 128 grid
    kw32 = [min(kw[g], max(32, -(-int(valid_lens[order[g]]) // 2) * 2))
            for g in range(len(order))]
    return order, kw, koff, int(koff[-1]), kw32


def _build_graph(valid_lens):
    _order, kws, koff, KEXT, kws32 = _kw_template(valid_lens)
    koff32 = np.concatenate([[0], np.cumsum(kws32)]).astype(int)
    KEXT32 = int(koff32[-1])
    nc = bacc.Bacc(
        "TRN2",
        target_bir_lowering=False,
        debug=False,
        num_devices=8,
    )

    # all inputs are host-packed p-major so every DMA reads long contiguous
    # runs per partition (the d-major layouts generated 512B descriptors
    # and ~8us completion latency on the critical first fetches)
    NB0 = DC * Q + 2 * DC * H + HT + DC * kws32[0]
    qT = nc.dram_tensor("boot0", [128, NB0], BF16, kind="ExternalInput")
    kT = nc.dram_tensor("kT", [128, DC * KEXT32], BF16, kind="ExternalInput")
    vext = nc.dram_tensor("vext", [128, (KEXT // 128) * VE], BF16,
                          kind="ExternalInput")
    # permutation matrices for the small-group score path: one [128,128]
    # one-hot matrix per (small group, chunk, half) mapping PSUM slab
    # quadrant rows {0,32,64,96} onto that block's 4 global slot rows
    small = []
    sidx = {g: i for i, g in enumerate(small)}
    NS = max(1, len(small))
    perm = nc.dram_tensor("perm", [128, NS * 4 * 128], F32,
                          kind="ExternalInput")
    out = nc.dram_tensor("out", [Q, V], F32, kind="ExternalOutput")

    KT = KEXT // 128  # 128-col k-tiles (template is 128-aligned)

    with tile.TileContext(nc) as tc, ExitStack() as ctx:
        singles = ctx.enter_context(tc.tile_pool(name="singles", bufs=1))
        work = ctx.enter_context(tc.tile_pool(name="work", bufs=2))
        psum = ctx.enter_context(tc.tile_pool(name="psum", bufs=1, space="PSUM"))

        # ---- load inputs ------------------------------------------------
        # Every DMA has a ~4-5us trigger->completion latency that
        # SERIALIZES per queue, so the whole boot-critical set (qT, wq,
        # wk, wv, kt0) is packed host-side into ONE dram tensor = one DMA
        # on sync; kt1 rides alone on the scalar queue.
        kw0, kw1 = kws32[0], kws32[1]
        boot0_sb = singles.tile([128, NB0], BF16)
        nc.sync.dma_start(boot0_sb[:], qT.ap())
        O_WQ = DC * Q
        O_WK = O_WQ + DC * H
        O_WV = O_WK + DC * H
        O_K0 = O_WV + HT

        def qt_r(dc):
            return boot0_sb[:, dc * Q : (dc + 1) * Q]

        def wq_r(dc, ht):
            o = O_WQ + dc * H + ht * 128
            return boot0_sb[:, o : o + 128]

        def wk_r(dc, ht):
            o = O_WK + dc * H + ht * 128
            return boot0_sb[:, o : o + 128]

        def wv_r(ht):
            return boot0_sb[:, O_WV + ht : O_WV + ht + 1]
        if small:
            perm_sb = singles.tile([128, NS, 4, 128], F32)
            nc.sync.dma_start(
                perm_sb[:],
                perm.ap().rearrange("p (s c k) -> p s c k", c=4, k=128),
            )

        ve_r = vext.ap().rearrange("p (t v) -> p t v", v=VE)  # [128, KT, VE]

        kt_tiles = {
            0: lambda dc, c0, cw: boot0_sb[:, O_K0 + dc * kw0 + c0 :
                                           O_K0 + dc * kw0 + c0 + cw],
        }

        def fetch_kt(g, eng=None):
            kw = kws32[g]
            off = int(koff32[g])
            t = work.tile([128, DC, max(kws32)], BF16, tag="ktc", bufs=2,
                          name=f"ktc{g}")
            (eng or nc.sync).dma_start(
                t[:, :, :kw],
                kT.ap()[:, DC * off : DC * (off + kw)].rearrange(
                    "p (c k) -> p c k", c=DC
                ),
            )
            kt_tiles[g] = (
                lambda tt: lambda dc, c0, cw: tt[:, dc, c0 : c0 + cw]
            )(t)

        vext_sb = singles.tile([128, KT, VE], BF16)

        def fetch_v(g):
            t0 = int(koff[g]) // 128
            t1 = t0 + kws[g] // 128
            nc.sync.dma_start(vext_sb[:, t0:t1, :], ve_r[:, t0:t1, :])

        fetch_kt(1, eng=nc.scalar)
        fetch_kt(2)

        # tiny warmup activation so the ~2.7us ACT table load overlaps the
        # input DMAs (issued after the scalar-queue DMA triggers so they
        # are not delayed behind the table load)
        warm = singles.tile([1, 2], F32)
        nc.vector.memset(warm[:], 0.0)
        nc.scalar.activation(warm[:], warm[:], mybir.ActivationFunctionType.Tanh)

        ident = singles.tile([128, 128], BF16)
        make_identity(nc, ident[:])

        # ---- projections ------------------------------------------------
        qh_sb = singles.tile([128, HT, Q], F32)
        kh_sb = singles.tile([128, HT, KEXT32], BF16)

        def _proj_q(ht):
            ps = psum.tile([128, 512], F32, tag="misc", bufs=2, name="ps")
            for dc in range(DC):
                nc.tensor.matmul(
                    ps[:, :Q],
                    lhsT=wq_r(dc, ht),
                    rhs=qt_r(dc),
                    start=(dc == 0),
                    stop=(dc == DC - 1),
                )
            nc.vector.tensor_copy(qh_sb[:, ht, :], ps[:, :Q])

        def _proj_k(g, ht, act=False):
            # act=True: PSUM->SBUF cast on the scalar engine -- used for
            # the ramp-phase prefetches (sections 0) where ACT idles while
            # the DVE serial chain gates the next group's first tanh
            kw, off = kws32[g], int(koff32[g])
            ktc = kt_tiles[g]
            for c0 in range(0, kw, 512):
                cw = min(512, kw - c0)
                ps = psum.tile([128, 512], F32, tag="misc", bufs=2, name="ps")
                for dc in range(DC):
                    nc.tensor.matmul(
                        ps[:, :cw],
                        lhsT=wk_r(dc, ht),
                        rhs=ktc(dc, c0, cw),
                        start=(dc == 0),
                        stop=(dc == DC - 1),
                    )
                if act:
                    nc.scalar.copy(
                        kh_sb[:, ht, off + c0 : off + c0 + cw], ps[:, :cw]
                    )
                else:
                    nc.vector.tensor_copy(
                        kh_sb[:, ht, off + c0 : off + c0 + cw], ps[:, :cw]
                    )

        # group 0's projections first so its adds/tanh start ASAP; qh ht0
        # before kh ht1 etc. so the very first adds have both operands
        _proj_q(0)
        _proj_k(0, 0)
        _proj_q(1)
        _proj_k(0, 1)

        # ---- scores + per-group epilogue --------------------------------
        av = psum.tile([128, 512], F32, tag="av", bufs=1)
        denom_sb = singles.tile([128, B], F32)
        sps_tiles = {}
        slabs = [
            psum.tile([128, 1024], F32, tag=f"slab{i}", name=f"slab{i}")
            for i in range(2)
        ]
        for s in slabs:
            nc.vector.memset(s[:], NEG)

        kt_state = {"done": 0}

        def _drain(kw, scores_t, row0, act=False):
            # act=True routes the PSUM->SBUF stage copies through the scalar
            # engine -- used where ACT would idle while DVE is the local
            # bottleneck (small-kw sections and the tail)
            for half in range(2):
                stage = work.tile(
                    [128, 1000], F32, tag="stage", bufs=2, name="stage"
                )
                if act:
                    nc.scalar.copy(stage[:, :kw], slabs[half][:, :kw])
                else:
                    nc.vector.tensor_copy(stage[:, :kw], slabs[half][:, :kw])
                nc.sync.dma_start(
                    scores_t[row0 + half * 4 : row0 + half * 4 + 4, :kw],
                    stage[0:128:32, :kw],
                )

        def _av_piece(g, exp_g, t0, t1, tail=False):
            base = int(koff[g]) // 128
            for t in range(t0, t1):
                kt = base + t
                expt = work.tile([128, 128], BF16, tag="expT", bufs=3, name="expt")
                pt = psum.tile([128, 128], BF16, tag="misc", bufs=2, name="pt")
                nc.tensor.transpose(
                    pt[:], exp_g[:, t * 128 : (t + 1) * 128], ident[:]
                )
                nc.vector.tensor_copy(expt[:], pt[:])
                nc.tensor.matmul(
                    av[:, 0:V],
                    lhsT=expt[:],
                    rhs=vext_sb[:, kt, 0:V],
                    start=(kt_state["done"] == 0),
                    stop=(kt_state["done"] == KT - 1),
                )
                kt_state["done"] += 1

        def _exp(g, scores_g):
            kw128 = kws[g]
            exp_g = work.tile([128, 1024], BF16, tag="exp", name="exp_g")
            if g in sidx:
                # small-group path: scores live in the sps PSUM bank (via
                # the permute matmuls); rows outside the group hold -60
                # (slab row 1 routed by the idx-0 perm matrix), so a
                # full-partition exp gives ~0 there. accum_out yields the
                # softmax denominators for free.
                kw = kws32[g]
                sps = sps_tiles.pop(g)
                if kw < kw128:
                    nc.gpsimd.memset(exp_g[:, kw:kw128], 0.0)
                nc.scalar.activation(
                    exp_g[:, :kw],
                    sps[:, :kw],
                    mybir.ActivationFunctionType.Exp,
                    accum_out=denom_sb[:, g : g + 1],
                )
            else:
                kw = kws32[g]
                if kw < kw128:
                    nc.gpsimd.memset(exp_g[:, kw:kw128], 0.0)
                nc.scalar.activation(
                    exp_g[:, :kw], scores_g[:, :kw],
                    mybir.ActivationFunctionType.Exp,
                    accum_out=denom_sb[:, g : g + 1],
                )
            return exp_g

        pending = None  # {"g","scores","exp","t"}

        def _pending_step(tail=False, exp_only=False):
            nonlocal pending
            if pending is None:
                return
            g_p = pending["g"]
            nt = kws[g_p] // 128
            if pending["exp"] is None:
                pending["exp"] = _exp(g_p, pending["scores"])
                if exp_only:
                    return
            t0 = pending["t"]
            t1 = nt if tail else min(nt, t0 + (nt + 1) // 2)
            if t1 > t0:
                _av_piece(g_p, pending["exp"], t0, t1, tail=tail)
            pending["t"] = t1
            if t1 >= nt:
                pending = None

        def _adds(g, jg, st2, gp_ht1=False):
            kw, off = kws32[g], int(koff32[g])
            for ht in range(HT):
                eng = nc.gpsimd if (gp_ht1 and ht == 1) else nc.vector
                for j in range(8):
                    slot = g * QPC + jg * 8 + j
                    eng.tensor_scalar_add(
                        st2[:, ht, j, :],
                        kh_sb[:, ht, off : off + kw],
                        qh_sb[:, ht, slot : slot + 1],
                    )

        def _scores_mm(g, jg, st2):
            # score matmuls: M=1 rows into PSUM slab quadrant rows
            # {0,32,64,96}; half 0 -> slab0, half 1 -> slab1
            kw = kws32[g]
            nchunk = (kw + 511) // 512
            for ht in range(HT):
                for half in range(2):
                    slab = slabs[half]
                    for j4 in range(4):
                        j = half * 4 + j4
                        for c in range(nchunk):
                            cw = min(512, kw - c * 512)
                            nc.tensor.matmul(
                                slab[
                                    32 * j4 : 32 * j4 + 1,
                                    c * 512 : c * 512 + cw,
                                ],
                                lhsT=wv_r(ht),
                                rhs=st2[:, ht, j, c * 512 : c * 512 + cw],
                                start=(ht == 0),
                                stop=(ht == 1),
                                tile_position=(0, 32 * j4),
                            )

        # software-pipelined section per group. Issue order is chosen so
        # the in-order DVE queue never puts work that depends on the
        # CURRENT tanh ahead of the adds feeding the NEXT tanh:
        #   [adds c0, adds c1, drain(g-1,c1), casts(g+2), drain(g,c0)]
        # and kproj runs on PE right after the first score matmuls, two
        # groups ahead of its use.
        def _perm_mm(g, jg):
            # permute the slab quadrant rows into the group's global slot
            # rows of av[:, 513:513+kw] via PE (stage copy on ACT, then a
            # one-hot matmul) -- replaces the strided-DMA drain for small
            # groups, avoiding the ~5us DMA round trip entirely
            kw = kws32[g]
            if jg == 0:
                sps_tiles[g] = psum.tile(
                    [128, 512], F32, tag="sps", bufs=1, name="sps"
                )
            for h in range(2):
                idx = jg * 2 + h
                stage = work.tile(
                    [128, 1000], F32, tag="stage", bufs=2, name="stage"
                )
                if h == 0:
                    nc.vector.tensor_copy(stage[:, :kw], slabs[h][:, :kw])
                else:
                    nc.scalar.copy(stage[:, :kw], slabs[h][:, :kw])
                nc.tensor.matmul(
                    sps_tiles[g][:, :kw],
                    lhsT=perm_sb[:, sidx[g], idx, :],
                    rhs=stage[:, :kw],
                    start=(idx == 0),
                    stop=(idx == 3),
                )

        prev_drain = None  # deferred last-chunk drain of the previous group
        for g in range(B):
            kw, off = kws32[g], int(koff[g])
            kw128 = kws[g]
            if g in sidx:
                scores_g = None
            else:
                scores_g = work.tile(
                    [128, 1000], F32, tag="scores", name="scores_g"
                )
                nc.gpsimd.memset(scores_g[:, :kw], NEG)
            st2a = work.tile([128, HT, 8, kw], BF16, tag="st", bufs=3, name="st2")
            st2b = work.tile([128, HT, 8, kw], BF16, tag="st", bufs=3, name="st2")
            _adds(g, 0, st2a)
            _adds(g, 1, st2b)
            if g == 0:
                _proj_k(1, 0, act=True)
                _proj_k(1, 1, act=True)
            if prev_drain is not None:
                # in small-kw sections DVE (adds) is the local bottleneck
                # while ACT idles -- shift the previous group's last drain
                # copies onto the scalar engine there
                _drain(*prev_drain, act=(kw <= 450))
                prev_drain = None
            # tanh chunk 0 (split at startup so ACT begins after 4 rows)
            if g == 0:
                for ht in range(HT):
                    nc.scalar.activation(
                        st2a[:, ht, 0:4, :], st2a[:, ht, 0:4, :],
                        mybir.ActivationFunctionType.Tanh,
                    )
                    nc.scalar.activation(
                        st2a[:, ht, 4:8, :], st2a[:, ht, 4:8, :],
                        mybir.ActivationFunctionType.Tanh,
                    )
            else:
                nc.scalar.activation(
                    st2a[:], st2a[:], mybir.ActivationFunctionType.Tanh
                )
            _scores_mm(g, 0, st2a)
            if g + 2 < B:
                _proj_k(g + 2, 0, act=(g == 0))
                _proj_k(g + 2, 1, act=(g == 0))
            if g + 3 < B:
                fetch_kt(g + 3)
            fetch_v(g)
            # in the last section, run only exp(g-1) here and defer its AV
            # matmuls until after this group's final score matmuls, so the
            # PE priority order doesn't delay the last drain
            _pending_step(exp_only=(g == B - 1))
            # drain chunk 0 before chunk 1's matmuls reuse the slabs
            if g in sidx:
                _perm_mm(g, 0)
            else:
                _drain(kw, scores_g, g * QPC)
            # tanh + scores chunk 1
            nc.scalar.activation(
                st2b[:], st2b[:], mybir.ActivationFunctionType.Tanh
            )
            _scores_mm(g, 1, st2b)
            if g in sidx:
                _perm_mm(g, 1)
            else:
                prev_drain = (kw, scores_g, g * QPC + 8)
            _pending_step(tail=(g == B - 1))
            pending = {"g": g, "scores": scores_g, "exp": None, "t": 0}
        if prev_drain is not None:
            _drain(*prev_drain, act=True)
        _pending_step(tail=True)
        _pending_step(tail=True)

        rl = singles.tile([128, 1], F32)
        dsum = singles.tile([128, 1], F32)
        nc.vector.reduce_sum(dsum[:], denom_sb[:], axis=mybir.AxisListType.X)
        nc.vector.reciprocal(rl[:], dsum[:])
        out_sb = singles.tile([128, V], F32)
        nc.vector.tensor_scalar_mul(out_sb[:], av[:, 0:V], rl[:])
        nc.sync.dma_start(out.ap(), out_sb[:])

    nc.compile()
    return nc


_CACHE = {}


def _install_profile_shim():
    """Provide antenv.axon_hooks (absent in this image) so
    run_bass_kernel_spmd(trace=True) can capture NTFF profiles through
    libaxon_pjrt.so, mirroring trn_agent_boot's bootstrap."""
    import types

    if "antenv.axon_hooks" not in sys.modules:
        mod = types.ModuleType("antenv.axon_hooks")
        state = {}
        mod.set_axon_ntff_profile_hook = lambda h: state.__setitem__("h", h)
        mod.get_axon_ntff_profile_hook = lambda: state.get("h")
        sys.modules["antenv.axon_hooks"] = mod
        import antenv

        antenv.axon_hooks = mod
        if "/root/.axon_site" not in sys.path:
            sys.path.insert(0, "/root/.axon_site")
        from trn_agent_boot.trn_boot import _ntff_profile_via_ctypes

        hook = _ntff_profile_via_ctypes("/opt/axon/libaxon_pjrt.so")
        mod.set_axon_ntff_profile_hook(hook)

        import concourse.bass_utils as bu

        orig_upload = bu.upload_artifacts

        def _safe_upload(tmpdir):
            try:
                return orig_upload(tmpdir)
            except Exception:
                return f"local:{tmpdir}"

        bu.upload_artifacts = _safe_upload


def _get_graph(valid_lens):
    key = tuple(int(v) for v in valid_lens)
    if _CACHE.get("key") != key:
        _CACHE["nc"] = _build_graph(valid_lens)
        _CACHE["key"] = key
    return _CACHE["nc"]


def _make_in_maps(queries, keys, values, valid_lens):
    order, kws, koff, KEXT, kws32 = _kw_template(valid_lens)
    koff32 = np.concatenate([[0], np.cumsum(kws32)]).astype(int)
    KEXT32 = int(koff32[-1])
    # p-major packed layouts: for each SBUF partition p, the data it will
    # receive is one contiguous run in DRAM (large DMA descriptors)
    kT = np.zeros((128, DC * KEXT32), dtype=np.float32)  # per-group blocks
    vext = np.zeros((KEXT // 128, 128, VE), dtype=np.float32)  # [t, p, v]
    for g, b in enumerate(order):
        vl = int(valid_lens[b])
        kw, off = kws32[g], int(koff32[g])
        # block for group g: kT[p, DC*off + c*kw + k] = keys[b, k, c*128+p]
        kb = keys[b, :kw].T.reshape(DC, 128, kw).transpose(1, 0, 2)
        kT[:, DC * off : DC * (off + kw)] = kb.reshape(128, DC * kw)
        off128 = int(koff[g])
        vext.reshape(KEXT, VE)[off128 : off128 + vl, :V] = values[b, :vl]
        vext.reshape(KEXT, VE)[off128 : off128 + vl, V] = 1.0
    kT_bf = kT.astype(NP_BF16)
    vext_bf = (
        vext.transpose(1, 0, 2).reshape(128, (KEXT // 128) * VE).copy()
        .astype(NP_BF16)
    )
    small = []
    NS = max(1, len(small))
    perm = np.zeros((128, NS, 4, 128), dtype=np.float32)
    for si, g in enumerate(small):
        # idx-0 matrix routes slab row 1 (never written, holds -60) into
        # every slot row outside this group, so exp gives ~0 there
        for j in range(128):
            if not (g * QPC <= j < (g + 1) * QPC):
                perm[1, si, 0, j] = 1.0
        for jg in range(2):
            for h in range(2):
                for j4 in range(4):
                    slot = g * QPC + jg * 8 + h * 4 + j4
                    perm[32 * j4, si, jg * 2 + h, slot] = 1.0
    perm_bf = perm.reshape(128, NS * 4 * 128).copy()
    in_maps = []
    for c in range(B):
        qrows = np.concatenate(
            [queries[b, c * QPC : (c + 1) * QPC] for b in order], axis=0
        )  # [128, D]; slot 16*g + r = (batch order[g], row 16*c + r)
        # qT[p, c, q] = qrows[q, c*128+p]
        qt = qrows.T.reshape(DC, 128, Q).transpose(1, 0, 2)
        boot0 = np.concatenate(
            [
                qt.reshape(128, DC * Q).astype(NP_BF16),
                _CACHE["wq_bf"],
                _CACHE["wk_bf"],
                _CACHE["wv2_bf"],
                kT_bf[:, : DC * kws32[0]],
            ],
            axis=1,
        )
        in_maps.append(
            {
                "boot0": np.ascontiguousarray(boot0),
                "kT": kT_bf,
                "vext": vext_bf,
                "perm": perm_bf,
            }
        )
    return in_maps


def kernel(
    queries, keys, values, valid_lens, Wq, Wk, wv, _profile=False, **_unused
):
    queries = np.asarray(queries, dtype=np.float32)
    keys = np.asarray(keys, dtype=np.float32)
    values = np.asarray(values, dtype=np.float32)
    valid_lens = np.asarray(valid_lens)
    def _pack_w(w):  # [D, H] -> [p, c*H] with w_p[p, c, h] = w[c*128+p, h]
        w = np.asarray(w, np.float32).reshape(DC, 128, H).transpose(1, 0, 2)
        return w.reshape(128, DC * H).copy().astype(NP_BF16)

    _CACHE["wq_bf"] = _pack_w(Wq)
    _CACHE["wk_bf"] = _pack_w(Wk)
    _CACHE["wv2_bf"] = (
        np.asarray(wv, np.float32).reshape(HT, 128).T.copy().astype(NP_BF16)
    )

    nc = _get_graph(valid_lens)
    in_maps = _make_in_maps(queries, keys, values, valid_lens)
    kwargs = {}
    if _profile:
        _install_profile_shim()
        tdir = "/root/problem/trace_out"
        os.makedirs(tdir, exist_ok=True)
        kwargs["tmpdir"] = tdir
    res = run_bass_kernel_spmd(
        nc, in_maps, core_ids=list(range(B)), trace=_profile, **kwargs
    )
    order = _kw_template(valid_lens)[0]
    out = np.zeros((B, Q, V), dtype=np.float32)
    for c in range(B):
        oc = np.asarray(res.results[c]["out"], dtype=np.float32)
        for g, b in enumerate(order):
            out[b, c * QPC : (c + 1) * QPC] = oc[g * QPC : (g + 1) * QPC]
    if _profile:
        _CACHE["last_result"] = res
    return out


# revision 66
# speedup vs baseline: 1.1597x; 1.1597x over previous
"""Additive attention (Bahdanau) Trainium2 kernel, SPMD over 8 NeuronCores.

Math per batch b (see reference):
    q = queries[b] @ Wq                  [Q=128, H=256]
    k = keys[b]    @ Wk                  [K=1024, H=256]
    scores[i,j] = sum_h wv[h] * tanh(q[i,h] + k[j,h])
    attn = masked_softmax(scores, valid_len[b])
    out[b] = attn @ values[b]            [Q, V=512]

Sharding: sequence-parallel q-striping. Each core takes 16 q-rows of EVERY
batch and only the valid k-range of each batch (rounded up to 128). Per-core
work = sum_b 16*ceil(vl_b/128)*128 columns -- perfectly balanced for any
valid_lens, no collectives (softmax is per-q-row and stays core-local).

Device pipeline (per core), h-on-partitions layout, fully group-streamed:
  - per group g (one batch): DMA its kT slice, project kh_g (PE) + cast
    (DVE), broadcast-add q (DVE tensor_scalar, bf16 4x), tanh both h-tiles
    in ONE fused ACTIVATE per 8-row chunk (ACT is the critical engine at
    1 elem/cycle/lane), score rows via M=1 matmuls into PSUM slabs
    (quadrant tile_position), drain slab->scores_sb via stage copy +
    strided DMA
  - group g-1's epilogue (exp -> DMA-xbar transpose -> AV matmul into a
    single PSUM accumulator) is interleaved into group g's chunks so the
    scalar engine never waits
  - scores_sb pre-filled with -60 (GPSIMD memset) so cross-batch cells
    exp to ~0; no max subtraction (|scores| <= sum|wv| ~ 13)
  - mask baked into values_ext: rows k >= vl zeroed, last column 1[k < vl]
    so attn @ values_ext also yields the softmax denominator
  - out = av * recip(l)
"""

import os
import sys

import numpy as np

for _p in ("/opt/trn_rl_repo", "/root/.axon_site/_ro/trn_rl_repo"):
    if os.path.isdir(_p) and _p not in sys.path:
        sys.path.insert(0, _p)

os.environ.setdefault("MYCRO_LOCAL_CACHE", "1")

import ml_dtypes  # noqa: E402
from contextlib import ExitStack  # noqa: E402

import concourse.bass as bass  # noqa: E402
import concourse.tile as tile  # noqa: E402
from concourse import bacc, mybir  # noqa: E402
from concourse.bass_utils import run_bass_kernel_spmd  # noqa: E402
from concourse.masks import make_identity  # noqa: E402

BF16 = mybir.dt.bfloat16
F32 = mybir.dt.float32
NP_BF16 = ml_dtypes.bfloat16

B, Q, K, D, H, V = 8, 128, 1024, 512, 256, 512
DC = D // 128   # 4 contraction tiles for the projections
HT = H // 128   # 2 h-tiles
QPC = Q // B    # 16 q-rows per (batch, core)
VE = V + 1      # values extended with a ones column (softmax denominator)
NEG = -60.0     # filler for never-written score cells; exp(-60) ~ 9e-27


def _kw_template(valid_lens):
    """Group order: 2nd-smallest kw first (fast pipeline start), the rest
    descending, smallest last (short epilogue tail). Returns per-GROUP
    (batch index, kw, koff); everything (kT/v packing, q slots, scores
    columns) uses this order."""
    kwb = [min(K, max(128, int(-(-int(v) // 128) * 128))) for v in valid_lens]
    by_kw = sorted(range(len(kwb)), key=lambda b: (kwb[b], b))
    order = [by_kw[1]] + sorted(by_kw[2:], key=lambda b: (-kwb[b], b)) + [by_kw[0]]
    kw = [kwb[b] for b in order]
    koff = np.concatenate([[0], np.cumsum(kw)]).astype(int)
    # compute-stage widths on a 2 grid (S-build/tanh/score matmuls only --
    # even so DVE 4x packing applies); the epilogue (exp/transpose/AV)
    # stays on the 128 grid
    kw32 = [min(kw[g], max(32, -(-int(valid_lens[order[g]]) // 2) * 2))
            for g in range(len(order))]
    return order, kw, koff, int(koff[-1]), kw32


def _build_graph(valid_lens):
    _order, kws, koff, KEXT, kws32 = _kw_template(valid_lens)
    koff32 = np.concatenate([[0], np.cumsum(kws32)]).astype(int)
    KEXT32 = int(koff32[-1])
    nc = bacc.Bacc(
        "TRN2",
        target_bir_lowering=False,
        debug=False,
        num_devices=8,
    )

    # all inputs are host-packed p-major so every DMA reads long contiguous
    # runs per partition (the d-major layouts generated 512B descriptors
    # and ~8us completion latency on the critical first fetches)
    NB0 = DC * Q + 2 * DC * H + HT + DC * kws32[0]
    qT = nc.dram_tensor("boot0", [128, NB0], BF16, kind="ExternalInput")
    kT = nc.dram_tensor("kT", [128, DC * KEXT32], BF16, kind="ExternalInput")
    vext = nc.dram_tensor("vext", [128, (KEXT // 128) * VE], BF16,
                          kind="ExternalInput")
    # permutation matrices for the small-group score path: one [128,128]
    # one-hot matrix per (small group, chunk, half) mapping PSUM slab
    # quadrant rows {0,32,64,96} onto that block's 4 global slot rows
    small = []
    sidx = {g: i for i, g in enumerate(small)}
    NS = max(1, len(small))
    perm = nc.dram_tensor("perm", [128, NS * 4 * 128], F32,
                          kind="ExternalInput")
    out = nc.dram_tensor("out", [Q, V], F32, kind="ExternalOutput")

    KT = KEXT // 128  # 128-col k-tiles (template is 128-aligned)

    with tile.TileContext(nc) as tc, ExitStack() as ctx:
        singles = ctx.enter_context(tc.tile_pool(name="singles", bufs=1))
        work = ctx.enter_context(tc.tile_pool(name="work", bufs=2))
        psum = ctx.enter_context(tc.tile_pool(name="psum", bufs=1, space="PSUM"))

        # ---- load inputs ------------------------------------------------
        # Every DMA has a ~4-5us trigger->completion latency that
        # SERIALIZES per queue, so the whole boot-critical set (qT, wq,
        # wk, wv, kt0) is packed host-side into ONE dram tensor = one DMA
        # on sync; kt1 rides alone on the scalar queue.
        kw0, kw1 = kws32[0], kws32[1]
        boot0_sb = singles.tile([128, NB0], BF16)
        nc.sync.dma_start(boot0_sb[:], qT.ap())
        O_WQ = DC * Q
        O_WK = O_WQ + DC * H
        O_WV = O_WK + DC * H
        O_K0 = O_WV + HT

        def qt_r(dc):
            return boot0_sb[:, dc * Q : (dc + 1) * Q]

        def wq_r(dc, ht):
            o = O_WQ + dc * H + ht * 128
            return boot0_sb[:, o : o + 128]

        def wk_r(dc, ht):
            o = O_WK + dc * H + ht * 128
            return boot0_sb[:, o : o + 128]

        def wv_r(ht):
            return boot0_sb[:, O_WV + ht : O_WV + ht + 1]
        if small:
            perm_sb = singles.tile([128, NS, 4, 128], F32)
            nc.sync.dma_start(
                perm_sb[:],
                perm.ap().rearrange("p (s c k) -> p s c k", c=4, k=128),
            )

        ve_r = vext.ap().rearrange("p (t v) -> p t v", v=VE)  # [128, KT, VE]

        kt_tiles = {
            0: lambda dc, c0, cw: boot0_sb[:, O_K0 + dc * kw0 + c0 :
                                           O_K0 + dc * kw0 + c0 + cw],
        }

        def fetch_kt(g, eng=None):
            kw = kws32[g]
            off = int(koff32[g])
            t = work.tile([128, DC, max(kws32)], BF16, tag="ktc", bufs=2,
                          name=f"ktc{g}")
            (eng or nc.sync).dma_start(
                t[:, :, :kw],
                kT.ap()[:, DC * off : DC * (off + kw)].rearrange(
                    "p (c k) -> p c k", c=DC
                ),
            )
            kt_tiles[g] = (
                lambda tt: lambda dc, c0, cw: tt[:, dc, c0 : c0 + cw]
            )(t)

        vext_sb = singles.tile([128, KT, VE], BF16)

        def fetch_v(g):
            t0 = int(koff[g]) // 128
            t1 = t0 + kws[g] // 128
            nc.sync.dma_start(vext_sb[:, t0:t1, :], ve_r[:, t0:t1, :])

        fetch_kt(1, eng=nc.scalar)
        fetch_kt(2, eng=nc.scalar)

        # tiny warmup activation so the ~2.7us ACT table load overlaps the
        # input DMAs (issued after the scalar-queue DMA triggers so they
        # are not delayed behind the table load)
        warm = singles.tile([1, 2], F32)
        nc.vector.memset(warm[:], 0.0)
        nc.scalar.activation(warm[:], warm[:], mybir.ActivationFunctionType.Tanh)

        ident = singles.tile([128, 128], BF16)
        make_identity(nc, ident[:])

        # ---- projections ------------------------------------------------
        qh_sb = singles.tile([128, HT, Q], F32)
        kh_sb = singles.tile([128, HT, KEXT32], BF16)

        def _proj_q(ht):
            ps = psum.tile([128, 512], F32, tag="misc", bufs=2, name="ps")
            for dc in range(DC):
                nc.tensor.matmul(
                    ps[:, :Q],
                    lhsT=wq_r(dc, ht),
                    rhs=qt_r(dc),
                    start=(dc == 0),
                    stop=(dc == DC - 1),
                )
            nc.vector.tensor_copy(qh_sb[:, ht, :], ps[:, :Q])

        def _proj_k(g, ht, act=False):
            # act=True: PSUM->SBUF cast on the scalar engine -- used for
            # the ramp-phase prefetches (sections 0) where ACT idles while
            # the DVE serial chain gates the next group's first tanh
            kw, off = kws32[g], int(koff32[g])
            ktc = kt_tiles[g]
            for c0 in range(0, kw, 512):
                cw = min(512, kw - c0)
                ps = psum.tile([128, 512], F32, tag="misc", bufs=2, name="ps")
                for dc in range(DC):
                    nc.tensor.matmul(
                        ps[:, :cw],
                        lhsT=wk_r(dc, ht),
                        rhs=ktc(dc, c0, cw),
                        start=(dc == 0),
                        stop=(dc == DC - 1),
                    )
                if act:
                    nc.scalar.copy(
                        kh_sb[:, ht, off + c0 : off + c0 + cw], ps[:, :cw]
                    )
                else:
                    nc.vector.tensor_copy(
                        kh_sb[:, ht, off + c0 : off + c0 + cw], ps[:, :cw]
                    )

        # group 0's projections first so its adds/tanh start ASAP; qh ht0
        # before kh ht1 etc. so the very first adds have both operands
        _proj_q(0)
        _proj_k(0, 0)
        _proj_q(1)
        _proj_k(0, 1)

        # ---- scores + per-group epilogue --------------------------------
        av = psum.tile([128, 512], F32, tag="av", bufs=1)
        denom_sb = singles.tile([128, B], F32)
        sps_tiles = {}
        slabs = [
            psum.tile([128, 1024], F32, tag=f"slab{i}", name=f"slab{i}")
            for i in range(2)
        ]
        for s in slabs:
            nc.vector.memset(s[:], NEG)

        kt_state = {"done": 0}

        def _drain(kw, scores_t, row0, act=False):
            # act=True routes the PSUM->SBUF stage copies through the scalar
            # engine -- used where ACT would idle while DVE is the local
            # bottleneck (small-kw sections and the tail)
            for half in range(2):
                stage = work.tile(
                    [128, 1000], F32, tag="stage", bufs=2, name="stage"
                )
                if act:
                    nc.scalar.copy(stage[:, :kw], slabs[half][:, :kw])
                else:
                    nc.vector.tensor_copy(stage[:, :kw], slabs[half][:, :kw])
                nc.sync.dma_start(
                    scores_t[row0 + half * 4 : row0 + half * 4 + 4, :kw],
                    stage[0:128:32, :kw],
                )

        def _av_piece(g, exp_g, t0, t1, tail=False):
            base = int(koff[g]) // 128
            for t in range(t0, t1):
                kt = base + t
                expt = work.tile([128, 128], BF16, tag="expT", bufs=3, name="expt")
                pt = psum.tile([128, 128], BF16, tag="misc", bufs=2, name="pt")
                nc.tensor.transpose(
                    pt[:], exp_g[:, t * 128 : (t + 1) * 128], ident[:]
                )
                nc.vector.tensor_copy(expt[:], pt[:])
                nc.tensor.matmul(
                    av[:, 0:V],
                    lhsT=expt[:],
                    rhs=vext_sb[:, kt, 0:V],
                    start=(kt_state["done"] == 0),
                    stop=(kt_state["done"] == KT - 1),
                )
                kt_state["done"] += 1

        def _exp(g, scores_g):
            kw128 = kws[g]
            exp_g = work.tile([128, 1024], BF16, tag="exp", name="exp_g")
            if g in sidx:
                # small-group path: scores live in the sps PSUM bank (via
                # the permute matmuls); rows outside the group hold -60
                # (slab row 1 routed by the idx-0 perm matrix), so a
                # full-partition exp gives ~0 there. accum_out yields the
                # softmax denominators for free.
                kw = kws32[g]
                sps = sps_tiles.pop(g)
                if kw < kw128:
                    nc.gpsimd.memset(exp_g[:, kw:kw128], 0.0)
                nc.scalar.activation(
                    exp_g[:, :kw],
                    sps[:, :kw],
                    mybir.ActivationFunctionType.Exp,
                    accum_out=denom_sb[:, g : g + 1],
                )
            else:
                kw = kws32[g]
                if kw < kw128:
                    nc.gpsimd.memset(exp_g[:, kw:kw128], 0.0)
                nc.scalar.activation(
                    exp_g[:, :kw], scores_g[:, :kw],
                    mybir.ActivationFunctionType.Exp,
                    accum_out=denom_sb[:, g : g + 1],
                )
            return exp_g

        pending = None  # {"g","scores","exp","t"}

        def _pending_step(tail=False, exp_only=False):
            nonlocal pending
            if pending is None:
                return
            g_p = pending["g"]
            nt = kws[g_p] // 128
            if pending["exp"] is None:
                pending["exp"] = _exp(g_p, pending["scores"])
                if exp_only:
                    return
            t0 = pending["t"]
            t1 = nt if tail else min(nt, t0 + (nt + 1) // 2)
            if t1 > t0:
                _av_piece(g_p, pending["exp"], t0, t1, tail=tail)
            pending["t"] = t1
            if t1 >= nt:
                pending = None

        def _adds(g, jg, st2, gp_ht1=False):
            kw, off = kws32[g], int(koff32[g])
            for ht in range(HT):
                eng = nc.gpsimd if (gp_ht1 and ht == 1) else nc.vector
                for j in range(8):
                    slot = g * QPC + jg * 8 + j
                    eng.tensor_scalar_add(
                        st2[:, ht, j, :],
                        kh_sb[:, ht, off : off + kw],
                        qh_sb[:, ht, slot : slot + 1],
                    )

        def _scores_mm(g, jg, st2):
            # score matmuls: M=1 rows into PSUM slab quadrant rows
            # {0,32,64,96}; half 0 -> slab0, half 1 -> slab1
            kw = kws32[g]
            nchunk = (kw + 511) // 512
            for ht in range(HT):
                for half in range(2):
                    slab = slabs[half]
                    for j4 in range(4):
                        j = half * 4 + j4
                        for c in range(nchunk):
                            cw = min(512, kw - c * 512)
                            nc.tensor.matmul(
                                slab[
                                    32 * j4 : 32 * j4 + 1,
                                    c * 512 : c * 512 + cw,
                                ],
                                lhsT=wv_r(ht),
                                rhs=st2[:, ht, j, c * 512 : c * 512 + cw],
                                start=(ht == 0),
                                stop=(ht == 1),
                                tile_position=(0, 32 * j4),
                            )

        # software-pipelined section per group. Issue order is chosen so
        # the in-order DVE queue never puts work that depends on the
        # CURRENT tanh ahead of the adds feeding the NEXT tanh:
        #   [adds c0, adds c1, drain(g-1,c1), casts(g+2), drain(g,c0)]
        # and kproj runs on PE right after the first score matmuls, two
        # groups ahead of its use.
        def _perm_mm(g, jg):
            # permute the slab quadrant rows into the group's global slot
            # rows of av[:, 513:513+kw] via PE (stage copy on ACT, then a
            # one-hot matmul) -- replaces the strided-DMA drain for small
            # groups, avoiding the ~5us DMA round trip entirely
            kw = kws32[g]
            if jg == 0:
                sps_tiles[g] = psum.tile(
                    [128, 512], F32, tag="sps", bufs=1, name="sps"
                )
            for h in range(2):
                idx = jg * 2 + h
                stage = work.tile(
                    [128, 1000], F32, tag="stage", bufs=2, name="stage"
                )
                if h == 0:
                    nc.vector.tensor_copy(stage[:, :kw], slabs[h][:, :kw])
                else:
                    nc.scalar.copy(stage[:, :kw], slabs[h][:, :kw])
                nc.tensor.matmul(
                    sps_tiles[g][:, :kw],
                    lhsT=perm_sb[:, sidx[g], idx, :],
                    rhs=stage[:, :kw],
                    start=(idx == 0),
                    stop=(idx == 3),
                )

        prev_drain = None  # deferred last-chunk drain of the previous group
        for g in range(B):
            kw, off = kws32[g], int(koff[g])
            kw128 = kws[g]
            if g in sidx:
                scores_g = None
            else:
                scores_g = work.tile(
                    [128, 1000], F32, tag="scores", name="scores_g"
                )
                nc.gpsimd.memset(scores_g[:, :kw], NEG)
            st2a = work.tile([128, HT, 8, kw], BF16, tag="st", bufs=3, name="st2")
            st2b = work.tile([128, HT, 8, kw], BF16, tag="st", bufs=3, name="st2")
            _adds(g, 0, st2a)
            _adds(g, 1, st2b)
            if g == 0:
                _proj_k(1, 0, act=True)
                _proj_k(1, 1, act=True)
            if prev_drain is not None:
                # in small-kw sections DVE (adds) is the local bottleneck
                # while ACT idles -- shift the previous group's last drain
                # copies onto the scalar engine there
                _drain(*prev_drain, act=(kw <= 450))
                prev_drain = None
            # tanh chunk 0 (split at startup so ACT begins after 4 rows)
            if g == 0:
                for ht in range(HT):
                    nc.scalar.activation(
                        st2a[:, ht, 0:4, :], st2a[:, ht, 0:4, :],
                        mybir.ActivationFunctionType.Tanh,
                    )
                    nc.scalar.activation(
                        st2a[:, ht, 4:8, :], st2a[:, ht, 4:8, :],
                        mybir.ActivationFunctionType.Tanh,
                    )
            else:
                nc.scalar.activation(
                    st2a[:], st2a[:], mybir.ActivationFunctionType.Tanh
                )
            _scores_mm(g, 0, st2a)
            if g + 2 < B:
                _proj_k(g + 2, 0, act=(g == 0))
                _proj_k(g + 2, 1, act=(g == 0))
            if g + 3 < B:
                fetch_kt(g + 3)
            fetch_v(g)
            # in the last section, run only exp(g-1) here and defer its AV
            # matmuls until after this group's final score matmuls, so the
            # PE priority order doesn't delay the last drain
            _pending_step(exp_only=(g == B - 1))
            # drain chunk 0 before chunk 1's matmuls reuse the slabs
            if g in sidx:
                _perm_mm(g, 0)
            else:
                _drain(kw, scores_g, g * QPC)
            # tanh + scores chunk 1
            nc.scalar.activation(
                st2b[:], st2b[:], mybir.ActivationFunctionType.Tanh
            )
            _scores_mm(g, 1, st2b)
            if g in sidx:
                _perm_mm(g, 1)
            else:
                prev_drain = (kw, scores_g, g * QPC + 8)
            _pending_step(tail=(g == B - 1))
            pending = {"g": g, "scores": scores_g, "exp": None, "t": 0}
        if prev_drain is not None:
            _drain(*prev_drain, act=True)
        _pending_step(tail=True)
        _pending_step(tail=True)

        rl = singles.tile([128, 1], F32)
        dsum = singles.tile([128, 1], F32)
        nc.vector.reduce_sum(dsum[:], denom_sb[:], axis=mybir.AxisListType.X)
        nc.vector.reciprocal(rl[:], dsum[:])
        out_sb = singles.tile([128, V], F32)
        nc.vector.tensor_scalar_mul(out_sb[:], av[:, 0:V], rl[:])
        nc.sync.dma_start(out.ap(), out_sb[:])

    nc.compile()
    return nc


_CACHE = {}


def _install_profile_shim():
    """Provide antenv.axon_hooks (absent in this image) so
    run_bass_kernel_spmd(trace=True) can capture NTFF profiles through
    libaxon_pjrt.so, mirroring trn_agent_boot's bootstrap."""
    import types

    if "antenv.axon_hooks" not in sys.modules:
        mod = types.ModuleType("antenv.axon_hooks")
        state = {}
        mod.set_axon_ntff_profile_hook = lambda h: state.__setitem__("h", h)
        mod.get_axon_ntff_profile_hook = lambda: state.get("h")
        sys.modules["antenv.axon_hooks"] = mod
        import antenv

        antenv.axon_hooks = mod
        if "/root/.axon_site" not in sys.path:
            sys.path.insert(0, "/root/.axon_site")
        from trn_agent_boot.trn_boot import _ntff_profile_via_ctypes

        hook = _ntff_profile_via_ctypes("/opt/axon/libaxon_pjrt.so")
        mod.set_axon_ntff_profile_hook(hook)

        import concourse.bass_utils as bu

        orig_upload = bu.upload_artifacts

        def _safe_upload(tmpdir):
            try:
                return orig_upload(tmpdir)
            except Exception:
                return f"local:{tmpdir}"

        bu.upload_artifacts = _safe_upload


def _get_graph(valid_lens):
    key = tuple(int(v) for v in valid_lens)
    if _CACHE.get("key") != key:
        _CACHE["nc"] = _build_graph(valid_lens)
        _CACHE["key"] = key
    return _CACHE["nc"]


def _make_in_maps(queries, keys, values, valid_lens):
    order, kws, koff, KEXT, kws32 = _kw_template(valid_lens)
    koff32 = np.concatenate([[0], np.cumsum(kws32)]).astype(int)
    KEXT32 = int(koff32[-1])
    # p-major packed layouts: for each SBUF partition p, the data it will
    # receive is one contiguous run in DRAM (large DMA descriptors)
    kT = np.zeros((128, DC * KEXT32), dtype=np.float32)  # per-group blocks
    vext = np.zeros((KEXT // 128, 128, VE), dtype=np.float32)  # [t, p, v]
    for g, b in enumerate(order):
        vl = int(valid_lens[b])
        kw, off = kws32[g], int(koff32[g])
        # block for group g: kT[p, DC*off + c*kw + k] = keys[b, k, c*128+p]
        kb = keys[b, :kw].T.reshape(DC, 128, kw).transpose(1, 0, 2)
        kT[:, DC * off : DC * (off + kw)] = kb.reshape(128, DC * kw)
        off128 = int(koff[g])
        vext.reshape(KEXT, VE)[off128 : off128 + vl, :V] = values[b, :vl]
        vext.reshape(KEXT, VE)[off128 : off128 + vl, V] = 1.0
    kT_bf = kT.astype(NP_BF16)
    vext_bf = (
        vext.transpose(1, 0, 2).reshape(128, (KEXT // 128) * VE).copy()
        .astype(NP_BF16)
    )
    small = []
    NS = max(1, len(small))
    perm = np.zeros((128, NS, 4, 128), dtype=np.float32)
    for si, g in enumerate(small):
        # idx-0 matrix routes slab row 1 (never written, holds -60) into
        # every slot row outside this group, so exp gives ~0 there
        for j in range(128):
            if not (g * QPC <= j < (g + 1) * QPC):
                perm[1, si, 0, j] = 1.0
        for jg in range(2):
            for h in range(2):
                for j4 in range(4):
                    slot = g * QPC + jg * 8 + h * 4 + j4
                    perm[32 * j4, si, jg * 2 + h, slot] = 1.0
    perm_bf = perm.reshape(128, NS * 4 * 128).copy()
    in_maps = []
    for c in range(B):
        qrows = np.concatenate(
            [queries[b, c * QPC : (c + 1) * QPC] for b in order], axis=0
        )  # [128, D]; slot 16*g + r = (batch order[g], row 16*c + r)
        # qT[p, c, q] = qrows[q, c*128+p]
        qt = qrows.T.reshape(DC, 128, Q).transpose(1, 0, 2)
        boot0 = np.concatenate(
            [
                qt.reshape(128, DC * Q).astype(NP_BF16),
                _CACHE["wq_bf"],
                _CACHE["wk_bf"],
                _CACHE["wv2_bf"],
                kT_bf[:, : DC * kws32[0]],
            ],
            axis=1,
        )
        in_maps.append(
            {
                "boot0": np.ascontiguousarray(boot0),
                "kT": kT_bf,
                "vext": vext_bf,
                "perm": perm_bf,
            }
        )
    return in_maps


def kernel(
    queries, keys, values, valid_lens, Wq, Wk, wv, _profile=False, **_unused
):
    queries = np.asarray(queries, dtype=np.float32)
    keys = np.asarray(keys, dtype=np.float32)
    values = np.asarray(values, dtype=np.float32)
    valid_lens = np.asarray(valid_lens)
    def _pack_w(w):  # [D, H] -> [p, c*H] with w_p[p, c, h] = w[c*128+p, h]
        w = np.asarray(w, np.float32).reshape(DC, 128, H).transpose(1, 0, 2)
        return w.reshape(128, DC * H).copy().astype(NP_BF16)

    _CACHE["wq_bf"] = _pack_w(Wq)
    _CACHE["wk_bf"] = _pack_w(Wk)
    _CACHE["wv2_bf"] = (
        np.asarray(wv, np.float32).reshape(HT, 128).T.copy().astype(NP_BF16)
    )

    nc = _get_graph(valid_lens)
    in_maps = _make_in_maps(queries, keys, values, valid_lens)
    kwargs = {}
    if _profile:
        _install_profile_shim()
        tdir = "/root/problem/trace_out"
        os.makedirs(tdir, exist_ok=True)
        kwargs["tmpdir"] = tdir
    res = run_bass_kernel_spmd(
        nc, in_maps, core_ids=list(range(B)), trace=_profile, **kwargs
    )
    order = _kw_template(valid_lens)[0]
    out = np.zeros((B, Q, V), dtype=np.float32)
    for c in range(B):
        oc = np.asarray(res.results[c]["out"], dtype=np.float32)
        for g, b in enumerate(order):
            out[b, c * QPC : (c + 1) * QPC] = oc[g * QPC : (g + 1) * QPC]
    if _profile:
        _CACHE["last_result"] = res
    return out
